# revision 1
# baseline (speedup 1.0000x reference)
"""Trainium2 Bass kernel for batched tanh-attention flat-softmax.

Per batch b:
    Q = query[b] @ W_query; K = query[b] @ W_key      # [S, 64]
    s = tanh(Q @ K.T) * 10                            # [S, S]
    s[diag] = -inf
    out[b] = softmax(s.flatten())

Sharding: data-parallel over batch across 8 NeuronCores (6 batches/core),
W_query/W_key replicated; no cross-core communication.

Numerics: tanh(x)*10 is bounded in [-10,10], so softmax needs no max
subtraction: out = exp(10*tanh(s)) / sum(...). The diagonal is clamped to
-1e4 on the tanh output, so exp underflows to exactly 0 (matching the
reference's additive -1e8 mask).

Precision strategy: all matmuls run in bf16 hi/lo split form (1 cyc/row on
PE vs 4 for fp32) with fp32 PSUM accumulation:
  - query is split once: q = qh + ql (bf16 pair, exact to ~2^-17)
  - queryT is built by hardware DMA-transpose of the bf16 halves (free)
  - projections: [Q;K] = [Wq|Wk]h.T qh + [Wq|Wk]h.T ql + [Wq|Wk]l.T qh
  - scores: [Qh;Ql].T [Kh;Kh] + Qh.T Kl  (packed into 128 partitions)
Dropped terms are O(2^-18) relative; measured end-to-end max elementwise
error vs the fp32 reference is ~2e-3 on tiny elements, L2 ~1e-5.
"""

import numpy as np

import concourse.bass as bass
import concourse.bass_isa as bass_isa
import concourse.mybir as mybir
import concourse.tile as tile
from concourse import bacc
from concourse.bass_utils import run_bass_kernel_spmd

B = 48
S = 1024
D = 128
DK = 64
N_CORES = 8
BPC = B // N_CORES
P = 128
NQ = S // P
F32 = mybir.dt.float32
BF16 = mybir.dt.bfloat16
AL = mybir.AluOpType

TANH_CLIP = 10.0
DIAG_NEG = -1.0e4


def build_bass() -> bass.Bass:
    nc = bacc.Bacc(None, target_bir_lowering=False)

    q_d = nc.dram_tensor("query", [BPC, S, D], F32, kind="ExternalInput")
    wq_d = nc.dram_tensor("W_query", [D, DK], F32, kind="ExternalInput")
    wk_d = nc.dram_tensor("W_key", [D, DK], F32, kind="ExternalInput")
    out_d = nc.dram_tensor("out", [BPC, S, S], F32, kind="ExternalOutput")

    with tile.TileContext(nc) as tc:
        with (
            tc.tile_pool(name="singles", bufs=1) as singles,
            tc.tile_pool(name="qload", bufs=2) as qload,
            tc.tile_pool(name="qtp", bufs=2) as qtp,
            tc.tile_pool(name="projsb", bufs=2) as projsb,
            tc.tile_pool(name="tbuf", bufs=3) as tbuf,
            tc.tile_pool(name="small", bufs=2) as small,
            tc.tile_pool(name="dram", bufs=2, space="DRAM") as dpool,
            tc.tile_pool(name="ps_sc", bufs=4, space="PSUM") as ps_sc,
        ):
            # --- one-time setup ---
            # diag clamp mask: min(t, dmask) forces diagonal to -1e4
            dmask = singles.tile([P, P], F32)
            nc.vector.memset(dmask, 3.0e38)
            nc.gpsimd.affine_select(
                out=dmask,
                in_=dmask,
                compare_op=AL.not_equal,
                fill=DIAG_NEG,
                base=0,
                pattern=[[-1, P]],
                channel_multiplier=1,
            )

            # W stacked [Wq | Wk] as fp32, then bf16 hi/lo
            w32 = singles.tile([D, 2 * DK], F32)
            nc.sync.dma_start(w32[:, 0:DK], wq_d[:, :])
            nc.sync.dma_start(w32[:, DK:2 * DK], wk_d[:, :])
            wh = singles.tile([D, 2 * DK], BF16)
            nc.vector.tensor_copy(wh, w32)
            wl = singles.tile([D, 2 * DK], BF16)
            nc.vector.tensor_tensor(wl, w32, wh, AL.subtract)

            # ---- software-pipelined batch loop --------------------------
            # Emission order is engine-aware so no engine's in-order queue
            # blocks another batch's ready work (esp. DVE: batch i+1's
            # operand prep must precede batch i's normalize).

            def load_and_transpose(b):
                """Load query[b], split bf16 hi/lo, DMA-transpose both.

                Plain copy DMAs go through SWDGE (gpsimd) so the Sync
                HWDGE ring only ever runs DMA_TRANSPOSE — avoids the
                xbar-mode-switch serialization between transfer kinds."""
                q_sb = qload.tile([P, NQ, D], F32, tag="q")
                nc.sync.dma_start(
                    q_sb, q_d[b].rearrange("(n p) d -> p n d", p=P)
                )
                qh_sb = qload.tile([P, NQ, D], BF16, tag="qh")
                nc.vector.tensor_copy(qh_sb, q_sb)
                ql_sb = qload.tile([P, NQ, D], BF16, tag="ql")
                nc.vector.tensor_tensor(ql_sb, q_sb, qh_sb, AL.subtract)

                qhT = qtp.tile([D, S], BF16, tag="qhT")
                qlT = qtp.tile([D, S], BF16, tag="qlT")
                for src, dst, tg in ((qh_sb, qhT, "h"), (ql_sb, qlT, "l")):
                    scratch = dpool.tile([S, D], BF16, tag="scr" + tg)
                    nc.sync.dma_start(
                        scratch.rearrange("(n p) d -> p n d", p=P), src
                    )
                    nc.sync.dma_start_transpose(dst, scratch)
                return qhT, qlT

            def proj(qhT, qlT):
                """[Q; K] = [Wq|Wk].T queryT in bf16 hi/lo, fp32 psum."""
                pp = ps_sc.tile([P, S], F32, tag="sc", name="pp")
                for h in range(2):
                    cols = slice(h * 512, (h + 1) * 512)
                    nc.tensor.matmul(
                        pp[:, cols], wh, qhT[:, cols], start=True, stop=False
                    )
                    nc.tensor.matmul(
                        pp[:, cols], wh, qlT[:, cols], start=False, stop=False
                    )
                    nc.tensor.matmul(
                        pp[:, cols], wl, qhT[:, cols], start=False, stop=True
                    )
                return pp

            def build_stacks(pp):
                """Split Q/K psum into bf16 hi/lo matmul operands."""
                hb = projsb.tile([P, S], BF16, tag="hb")   # [Qh; Kh]
                nc.vector.tensor_copy(hb, pp)
                lb = projsb.tile([P, S], BF16, tag="lb")   # [Ql; Kl]
                nc.vector.tensor_tensor(lb, pp, hb, AL.subtract)

                qstack = projsb.tile([P, S], BF16, tag="qstack")  # [Qh; Ql]
                nc.vector.tensor_copy(qstack[0:DK], hb[0:DK])
                nc.vector.tensor_copy(qstack[DK:P], lb[0:DK])
                khh = projsb.tile([P, S], BF16, tag="khh")        # [Kh; Kh]
                nc.vector.tensor_copy(khh[0:DK], hb[DK:P])
                nc.vector.tensor_copy(khh[DK:P], hb[DK:P])
                # duplicated stacks for row-group-packed correction matmuls
                qhh = projsb.tile([P, S], BF16, tag="qhh")        # [Qh; Qh]
                nc.vector.tensor_copy(qhh[0:DK], hb[0:DK])
                nc.vector.tensor_copy(qhh[DK:P], hb[0:DK])
                kll = projsb.tile([P, S], BF16, tag="kll")        # [Kl; Kl]
                nc.vector.tensor_copy(kll[0:DK], lb[DK:P])
                nc.vector.tensor_copy(kll[DK:P], lb[DK:P])
                return qstack, khh, qhh, kll

            def scores_pair(t_sb, qstack, khh, qhh, kll, j):
                """Two 128-row score chunks (qc=2j, 2j+1), one 2-bank PSUM
                tile each. Main matmuls are 128-contraction; the two 64-
                contraction Qh.T Kl corrections run CONCURRENTLY in
                different PE row groups via tile_position."""
                qc0, qc1 = 2 * j, 2 * j + 1
                sc0 = ps_sc.tile([P, S], F32, tag="sc", name="sc0")
                sc1 = ps_sc.tile([P, S], F32, tag="sc", name="sc1")
                sl0 = slice(qc0 * P, (qc0 + 1) * P)
                sl1 = slice(qc1 * P, (qc1 + 1) * P)
                for h in range(2):
                    cols = slice(h * 512, (h + 1) * 512)
                    nc.tensor.matmul(
                        sc0[:, cols], qstack[:, sl0], khh[:, cols],
                        start=True, stop=False,
                    )
                    nc.tensor.matmul(
                        sc1[:, cols], qstack[:, sl1], khh[:, cols],
                        start=True, stop=False,
                    )
                    nc.tensor.matmul(
                        sc0[:, cols], qhh[0:DK, sl0], kll[0:DK, cols],
                        start=False, stop=True, tile_position=(0, 0),
                    )
                    nc.tensor.matmul(
                        sc1[:, cols], qhh[DK:P, sl1], kll[DK:P, cols],
                        start=False, stop=True, tile_position=(DK, 0),
                    )
                nc.scalar.activation(
                    out=t_sb[:, qc0],
                    in_=sc0,
                    func=mybir.ActivationFunctionType.Tanh,
                )
                nc.scalar.activation(
                    out=t_sb[:, qc1],
                    in_=sc1,
                    func=mybir.ActivationFunctionType.Tanh,
                )
                # clamp both diagonal blocks with one strided DVE op;
                # block qc sits at free offset qc*(S+P) with length P
                blk0 = t_sb[:, qc0, qc0 * P:(qc0 + 1) * P]
                diag_ap = bass.AP(
                    tensor=blk0.tensor,
                    offset=blk0.offset,
                    ap=[blk0.ap[0], [S + P, 2], [1, P]],
                )
                m0 = dmask[:, 0:P]
                mask_ap = bass.AP(
                    tensor=m0.tensor,
                    offset=m0.offset,
                    ap=[m0.ap[0], [0, 2], [1, P]],
                )
                nc.vector.tensor_tensor(diag_ap, diag_ap, mask_ap, AL.min)

            def exp_half(t_sb, rs, hidx):
                """exp(10*t) in place over half the batch rows, row sums
                accumulated into rs[:, hidx]."""
                nc.scalar.activation(
                    out=t_sb[:, 4 * hidx:4 * hidx + 4],
                    in_=t_sb[:, 4 * hidx:4 * hidx + 4],
                    func=mybir.ActivationFunctionType.Exp,
                    scale=TANH_CLIP,
                    accum_out=rs[:, hidx:hidx + 1],
                )

            def finish_batch(rs):
                """Z from the two half-sums; rz = 1/Z on all partitions."""
                zall = small.tile([P, 2], F32, tag="zall")
                nc.gpsimd.partition_all_reduce(
                    zall, rs, channels=P, reduce_op=bass_isa.ReduceOp.add
                )
                zsum = small.tile([P, 1], F32, tag="zsum")
                nc.vector.tensor_tensor(
                    zsum, zall[:, 0:1], zall[:, 1:2], AL.add
                )
                rz = small.tile([P, 1], F32, tag="rz")
                nc.vector.reciprocal(rz, zsum)
                return rz

            def store_batch(b, t_sb, rz):
                nc.vector.tensor_scalar_mul(t_sb, t_sb, rz)
                # big store via SWDGE on the (otherwise idle) GpSimd queue:
                # its wait-for-normalize must not block the SP ring's loads
                # and transposes, nor any compute engine's queue
                nc.gpsimd.dma_start(
                    out_d[b].rearrange("(n p) s -> p n s", p=P), t_sb
                )

            # prologue
            qhT, qlT = load_and_transpose(0)
            pp = proj(qhT, qlT)
            ops = build_stacks(pp)
            pending = None  # (b, t_sb, rz) awaiting normalize+store

            for b in range(BPC):
                t_sb = tbuf.tile([P, NQ, S], F32, tag="t")
                rs = small.tile([P, 2], F32, tag="rs")

                if pending is not None:
                    store_batch(*pending)
                    pending = None
                if b + 1 < BPC:
                    # start next batch's load/split/transpose chain early;
                    # it needs ~10us of DMA latency to land
                    nqhT, nqlT = load_and_transpose(b + 1)

                scores_pair(t_sb, *ops, 0)
                scores_pair(t_sb, *ops, 1)
                exp_half(t_sb, rs, 0)
                scores_pair(t_sb, *ops, 2)

                if b + 1 < BPC:
                    npp = proj(nqhT, nqlT)
                    nops = build_stacks(npp)

                scores_pair(t_sb, *ops, 3)
                if b + 1 < BPC:
                    ops = nops

                exp_half(t_sb, rs, 1)
                rz = finish_batch(rs)
                pending = (b, t_sb, rz)

            store_batch(*pending)

    nc.compile()
    return nc


_CACHED_NC = None


def kernel(**inputs: np.ndarray) -> np.ndarray:
    global _CACHED_NC
    query = np.ascontiguousarray(np.asarray(inputs["query"], dtype=np.float32))
    wq = np.ascontiguousarray(np.asarray(inputs["W_query"], dtype=np.float32))
    wk = np.ascontiguousarray(np.asarray(inputs["W_key"], dtype=np.float32))
    assert query.shape == (B, S, D), query.shape

    if _CACHED_NC is None:
        _CACHED_NC = build_bass()
    nc = _CACHED_NC

    in_maps = [
        {
            "query": query[c * BPC:(c + 1) * BPC],
            "W_query": wq,
            "W_key": wk,
        }
        for c in range(N_CORES)
    ]
    res = run_bass_kernel_spmd(nc, in_maps, core_ids=list(range(N_CORES)))
    out = np.concatenate(
        [r["out"].reshape(BPC, S * S) for r in res.results], axis=0
    )
    return out



# revision 2
# speedup vs baseline: 1.0625x; 1.0625x over previous
"""Trainium2 Bass kernel for batched tanh-attention flat-softmax.

Per batch b:
    Q = query[b] @ W_query; K = query[b] @ W_key      # [S, 64]
    s = tanh(Q @ K.T) * 10                            # [S, S]
    s[diag] = -inf
    out[b] = softmax(s.flatten())

Sharding: data-parallel over batch across 8 NeuronCores (6 batches/core),
W_query/W_key replicated; no cross-core communication.

Numerics: tanh(x)*10 is bounded in [-10,10], so softmax needs no max
subtraction: out = exp(10*tanh(s)) / sum(...). The diagonal is clamped to
-30000 on the PSUM scores BEFORE tanh, so tanh saturates to -1 and
exp gives e^-10 ~ 4.5e-5 (vs the reference's exact 0); the L2 impact is
~1e-10 - far below tolerance - and it removes the DVE step between the
last tanh and the exp, keeping the Scalar engine 100% busy.

Precision strategy (validated vs fp64 reference: rel L2 ~ 1.2e-3):
  - query cast to a single fp16 during the DMA load (SWDGE cast, free)
  - queryT built by TensorE transposes (8x [128,128] fp16, no DRAM
    round trip: saves 6MB of HBM traffic vs the DMA-transpose approach)
  - W_query|W_key stacked, single fp16
  - proj: [Q;K].T = W.T @ qT, one fp16 matmul per 512-col window
  - Q/K split fp16 hi/lo from fp32 PSUM; scores = [Qh;Ql].T @ [Kh;Kh]
    (one 128-contraction matmul per window; Q@Kl term dropped, its
    relative size is ~2^-12)

Engine budget per batch (target: ScalarE-bound at ~15.2us/batch):
  ScalarE: 8x tanh [128,1024] (997ns) + 1x exp [128,8192] (7013ns)
  PE:      8 transposes + 2 proj + 16 score matmuls ~ 10.5us
  DVE:     diag clamps, hi/lo split, stacks, 4x normalize chunks ~ 11us
  DMA:     25.2MB out + 3MB in = 28.2MB ~ 79us total (HBM-bound)
"""

import numpy as np

import concourse.bass as bass
import concourse.bass_isa as bass_isa
import concourse.mybir as mybir
import concourse.tile as tile
from concourse import bacc
from concourse.bass_utils import run_bass_kernel_spmd

B = 48
S = 1024
D = 128
DK = 64
N_CORES = 8
BPC = B // N_CORES
P = 128
NQ = S // P
F32 = mybir.dt.float32
F16 = mybir.dt.float16
AL = mybir.AluOpType

TANH_CLIP = 10.0
DIAG_NEG = -30000.0


def build_bass() -> bass.Bass:
    nc = bacc.Bacc(None, target_bir_lowering=False)

    q_d = nc.dram_tensor("query", [BPC, S, D], F32, kind="ExternalInput")
    wq_d = nc.dram_tensor("W_query", [D, DK], F32, kind="ExternalInput")
    wk_d = nc.dram_tensor("W_key", [D, DK], F32, kind="ExternalInput")
    out_d = nc.dram_tensor("out", [BPC, S, S], F32, kind="ExternalOutput")

    with tile.TileContext(nc) as tc:
        with (
            tc.tile_pool(name="singles", bufs=1) as singles,
            tc.tile_pool(name="qload", bufs=2) as qload,
            tc.tile_pool(name="qtp", bufs=2) as qtp,
            tc.tile_pool(name="projsb", bufs=2) as projsb,
            tc.tile_pool(name="tbuf", bufs=2) as tbuf,
            tc.tile_pool(name="obuf", bufs=2) as obuf,
            tc.tile_pool(name="small", bufs=2) as small,
            tc.tile_pool(name="ps_qt", bufs=1, space="PSUM") as ps_qt,
            tc.tile_pool(name="ps_pp", bufs=1, space="PSUM") as ps_pp,
            tc.tile_pool(name="ps_sc", bufs=2, space="PSUM") as ps_sc,
        ):
            # --- one-time setup ---
            # diag clamp mask (fp32, for PSUM scores): min(s, dmask) forces
            # the diagonal to -30000 so tanh saturates to -1.
            dmask = singles.tile([P, P], F32)
            nc.vector.memset(dmask, 3.0e38)
            nc.gpsimd.affine_select(
                out=dmask,
                in_=dmask,
                compare_op=AL.not_equal,
                fill=DIAG_NEG,
                base=0,
                pattern=[[-1, P]],
                channel_multiplier=1,
            )
            # fp16 identity for TensorE transposes
            ident32 = singles.tile([P, P], F32)
            nc.vector.memset(ident32, 0.0)
            nc.gpsimd.affine_select(
                out=ident32,
                in_=ident32,
                compare_op=AL.not_equal,
                fill=1.0,
                base=0,
                pattern=[[-1, P]],
                channel_multiplier=1,
            )
            ident = singles.tile([P, P], F16)
            nc.vector.tensor_copy(ident, ident32)

            # warm the ACT table set (exp_and_others holds tanh+exp) while
            # the first query load is still in flight
            actwarm = singles.tile([P, 1], F32)
            nc.scalar.activation(
                out=actwarm, in_=ident32[:, 0:1],
                func=mybir.ActivationFunctionType.Tanh,
            )

            # W stacked [Wq | Wk] as fp32, cast to a single fp16
            w32 = singles.tile([D, 2 * DK], F32)
            nc.sync.dma_start(w32[:, 0:DK], wq_d[:, :])
            nc.sync.dma_start(w32[:, DK:2 * DK], wk_d[:, :])
            w16 = singles.tile([D, 2 * DK], F16)
            nc.vector.tensor_copy(w16, w32)

            # ---- per-batch stages -------------------------------------
            def load(b):
                """Cast-DMA query[b] fp32->fp16 straight into SBUF."""
                q16 = qload.tile([P, NQ, D], F16, tag="q16")
                nc.gpsimd.dma_start(
                    q16, q_d[b].rearrange("(n p) d -> p n d", p=P)
                )
                return q16

            def transpose(q16):
                """qT[d, 128n+p] = q16[p, n, d] via 8 TensorE transposes."""
                qtps = ps_qt.tile([P, S], F16, tag="qt", name="qtps")
                for n in range(NQ):
                    nc.tensor.transpose(
                        qtps[:, n * P:(n + 1) * P], q16[:, n, :], ident
                    )
                qT = qtp.tile([P, S], F16, tag="qT")
                nc.vector.tensor_copy(qT, qtps)
                return qT

            def proj(qT):
                """[Q;K].T = [Wq|Wk].T @ qT, fp16 x fp16 -> fp32 PSUM."""
                pp = ps_pp.tile([P, S], F32, tag="pp", name="pp")
                for h in range(2):
                    cols = slice(h * 512, (h + 1) * 512)
                    nc.tensor.matmul(
                        pp[:, cols], w16, qT[:, cols], start=True, stop=True
                    )
                return pp

            def build_stacks(pp):
                """fp16 hi/lo of Q, hi of K, stacked for packed matmuls."""
                hb = projsb.tile([P, S], F16, tag="hb")    # [Qh; Kh]
                nc.vector.tensor_copy(hb, pp)
                lb = projsb.tile([DK, S], F16, tag="lb")   # Ql
                nc.vector.tensor_tensor(lb, pp[0:DK], hb[0:DK], AL.subtract)

                qstack = projsb.tile([P, S], F16, tag="qstack")  # [Qh; Ql]
                nc.vector.tensor_copy(qstack[0:DK], hb[0:DK])
                nc.vector.tensor_copy(qstack[DK:P], lb)
                khh = projsb.tile([P, S], F16, tag="khh")        # [Kh; Kh]
                nc.vector.tensor_copy(khh[0:DK], hb[DK:P])
                nc.vector.tensor_copy(khh[DK:P], hb[DK:P])
                return qstack, khh

            def score_chunk(t16, qstack, khh, c):
                """One 128-row score chunk: matmuls, diag clamp, tanh."""
                sc = ps_sc.tile([P, S], F32, tag="sc", name="sc")
                for h in range(2):
                    cols = slice(h * 512, (h + 1) * 512)
                    nc.tensor.matmul(
                        sc[:, cols], qstack[:, c * P:(c + 1) * P],
                        khh[:, cols], start=True, stop=True,
                    )
                # clamp this chunk's diagonal block on PSUM, pre-tanh
                blk = sc[:, c * P:(c + 1) * P]
                nc.vector.tensor_tensor(blk, blk, dmask, AL.min)
                nc.scalar.activation(
                    out=t16[:, c], in_=sc,
                    func=mybir.ActivationFunctionType.Tanh,
                )

            def exp_batch(t16, o32, rs):
                nc.scalar.activation(
                    out=o32, in_=t16,
                    func=mybir.ActivationFunctionType.Exp,
                    scale=TANH_CLIP,
                    accum_out=rs,
                )

            def zrecip(rs):
                zall = small.tile([P, 1], F32, tag="zall")
                nc.gpsimd.partition_all_reduce(
                    zall, rs, channels=P, reduce_op=bass_isa.ReduceOp.add
                )
                rz = small.tile([P, 1], F32, tag="rz")
                nc.vector.reciprocal(rz, zall)
                return rz

            def norm_store(b, o32, rz):
                """Normalize + store in 4 chunks so the store streams early."""
                ov = out_d[b].rearrange("(n p) s -> p n s", p=P)
                for g in range(4):
                    sl = slice(2 * g, 2 * g + 2)
                    nc.vector.tensor_scalar_mul(o32[:, sl], o32[:, sl], rz)
                    nc.sync.dma_start(ov[:, sl], o32[:, sl])

            # ---- software-pipelined batch loop ------------------------
            # Engine-queue emission order is chosen so no in-order queue
            # blocks another batch's ready work:
            #   ACT: tanh_b x8, exp_b, tanh_{b+1} x8, ...   (always busy)
            #   PE:  scores_b, transposes_{b+1}, proj_{b+1}, scores_{b+1}
            #   DVE: norm_{b-1}, dmins_b, qtcopy_{b+1}, stacks_{b+1}, recip_b
            q16 = load(0)
            qT = transpose(q16)
            pp = proj(qT)
            ops = build_stacks(pp)
            pending = None  # (b, o32, rz) awaiting normalize+store

            for b in range(BPC):
                t16 = tbuf.tile([P, NQ, S], F16, tag="t16")
                o32 = obuf.tile([P, NQ, S], F32, tag="o32")
                rs = small.tile([P, 1], F32, tag="rs")

                if b + 1 < BPC:
                    nq16 = load(b + 1)
                if pending is not None:
                    norm_store(*pending)
                    pending = None

                for c in range(NQ):
                    score_chunk(t16, *ops, c)

                if b + 1 < BPC:
                    nqT = transpose(nq16)
                    npp = proj(nqT)
                    nops = build_stacks(npp)
                    ops = nops

                exp_batch(t16, o32, rs)
                rz = zrecip(rs)
                pending = (b, o32, rz)

            norm_store(*pending)

    nc.compile()
    return nc


_CACHED_NC = None


def kernel(**inputs: np.ndarray) -> np.ndarray:
    global _CACHED_NC
    query = np.ascontiguousarray(np.asarray(inputs["query"], dtype=np.float32))
    wq = np.ascontiguousarray(np.asarray(inputs["W_query"], dtype=np.float32))
    wk = np.ascontiguousarray(np.asarray(inputs["W_key"], dtype=np.float32))
    assert query.shape == (B, S, D), query.shape

    if _CACHED_NC is None:
        _CACHED_NC = build_bass()
    nc = _CACHED_NC

    in_maps = [
        {
            "query": query[c * BPC:(c + 1) * BPC],
            "W_query": wq,
            "W_key": wk,
        }
        for c in range(N_CORES)
    ]
    res = run_bass_kernel_spmd(nc, in_maps, core_ids=list(range(N_CORES)))
    out = np.concatenate(
        [r["out"].reshape(BPC, S * S) for r in res.results], axis=0
    )
    return out


# revision 3
# speedup vs baseline: 1.3246x; 1.2467x over previous
"""Trainium2 Bass kernel for batched tanh-attention flat-softmax.

Per batch b:
    Q = query[b] @ W_query; K = query[b] @ W_key      # [S, 64]
    s = tanh(Q @ K.T) * 10                            # [S, S]
    s[diag] = -inf
    out[b] = softmax(s.flatten())

Sharding: data-parallel over batch across 8 NeuronCores (6 batches/core),
W_query/W_key replicated; no cross-core communication.

Numerics: tanh(x)*10 is bounded in [-10,10], so softmax needs no max
subtraction: out = exp(10*tanh(s)) / sum(...). The diagonal gets -30000
accumulated INTO the PSUM scores by a tiny extra matmul (identity
stationary x (-30000*I) moving), so tanh saturates to -1 and exp gives
e^-10 ~ 4.5e-5 (vs the reference's exact 0); the L2 impact is ~1e-10.
This keeps the diag handling entirely on the PE - nothing sits between
the score matmuls and tanh, so the Scalar engine never stalls.

Precision strategy (validated vs fp64 reference: rel L2 ~ 1.2e-3):
  - query cast to a single fp16 during the DMA load (SWDGE cast, free)
  - queryT built by TensorE transposes (8x [128,128] fp16; no DRAM
    round trip: saves 6MB of HBM traffic vs a DMA-transpose approach)
  - W_query|W_key stacked, single fp16
  - proj: [Q;K].T = W.T @ qT, one fp16 matmul per 512-col window
  - Q split fp16 hi/lo from fp32 PSUM; scores = [Qh;Ql].T @ [Kh;Kh]
    (one 128-contraction matmul per window; Q@Kl term dropped, ~2^-12)

Engine budget per batch (ScalarE-bound at ~15.4us/batch):
  ScalarE: 8x tanh [128,1024] (~1030ns) + 1x exp [128,8192] (~7010ns)
  PE:      8 transposes + 2 proj + 16 score + 8 diag matmuls ~ 11us
  DVE:     qT copy, hi/lo split, stacks, 4x normalize chunks ~ 9us
  DMA:     25.2MB out + 3MB in = 28.2MB ~ 79us total (HBM-bound)
The per-stage work is window-split (2x 512 cols) so the first score
matmul of a batch only waits on half the operand-prep chain.
"""

import numpy as np

import concourse.bass as bass
import concourse.bass_isa as bass_isa
import concourse.mybir as mybir
import concourse.tile as tile
from concourse import bacc
from concourse.bass_utils import run_bass_kernel_spmd

B = 48
S = 1024
D = 128
DK = 64
N_CORES = 8
BPC = B // N_CORES
P = 128
NQ = S // P
F32 = mybir.dt.float32
F16 = mybir.dt.float16
AL = mybir.AluOpType

TANH_CLIP = 10.0
DIAG_NEG = -30000.0


def build_bass() -> bass.Bass:
    nc = bacc.Bacc(None, target_bir_lowering=False)

    q_d = nc.dram_tensor("query", [BPC, S, D], F32, kind="ExternalInput")
    wq_d = nc.dram_tensor("W_query", [D, DK], F32, kind="ExternalInput")
    wk_d = nc.dram_tensor("W_key", [D, DK], F32, kind="ExternalInput")
    out_d = nc.dram_tensor("out", [BPC, S, S], F32, kind="ExternalOutput")

    with tile.TileContext(nc) as tc:
        with (
            tc.tile_pool(name="singles", bufs=1) as singles,
            tc.tile_pool(name="qload", bufs=2) as qload,
            tc.tile_pool(name="qtp", bufs=2) as qtp,
            tc.tile_pool(name="projsb", bufs=2) as projsb,
            tc.tile_pool(name="tbuf", bufs=2) as tbuf,
            tc.tile_pool(name="obuf", bufs=3) as obuf,
            tc.tile_pool(name="small", bufs=2) as small,
            tc.tile_pool(name="ps_qt", bufs=1, space="PSUM") as ps_qt,
            tc.tile_pool(name="ps_sc", bufs=3, space="PSUM") as ps_sc,
        ):
            # --- one-time setup ---
            # fp16 identity (TensorE transposes + diag-accumulate stationary)
            ident32 = singles.tile([P, P], F32)
            nc.vector.memset(ident32, 0.0)
            nc.gpsimd.affine_select(
                out=ident32,
                in_=ident32,
                compare_op=AL.not_equal,
                fill=1.0,
                base=0,
                pattern=[[-1, P]],
                channel_multiplier=1,
            )
            ident = singles.tile([P, P], F16)
            nc.vector.tensor_copy(ident, ident32)
            # -30000 * I, the moving operand of the diag-accumulate matmul
            negd32 = singles.tile([P, P], F32)
            nc.vector.memset(negd32, 0.0)
            nc.gpsimd.affine_select(
                out=negd32,
                in_=negd32,
                compare_op=AL.not_equal,
                fill=DIAG_NEG,
                base=0,
                pattern=[[-1, P]],
                channel_multiplier=1,
            )
            negd = singles.tile([P, P], F16)
            nc.vector.tensor_copy(negd, negd32)

            # warm the ACT table set (exp_and_others holds tanh+exp) while
            # the first query load is still in flight
            actwarm = singles.tile([P, 1], F32)
            nc.scalar.activation(
                out=actwarm, in_=ident32[:, 0:1],
                func=mybir.ActivationFunctionType.Tanh,
            )

            # W stacked [Wq | Wk] as fp32, cast to a single fp16
            w32 = singles.tile([D, 2 * DK], F32)
            nc.sync.dma_start(w32[:, 0:DK], wq_d[:, :])
            nc.sync.dma_start(w32[:, DK:2 * DK], wk_d[:, :])
            w16 = singles.tile([D, 2 * DK], F16)
            nc.vector.tensor_copy(w16, w32)

            # ---- per-batch stages (window-split where useful) ----------
            def load(b):
                """Cast-DMA query[b] fp32->fp16 into SBUF, in two halves
                so the first transposes can start after half the data."""
                q16 = qload.tile([P, NQ, D], F16, tag="q16")
                hv = q_d[b].rearrange("(h n p) d -> h p n d", h=2, p=P)
                for h in range(2):
                    nc.gpsimd.dma_start(q16[:, 4 * h:4 * h + 4], hv[h])
                return q16

            def transpose_win(q16, h):
                """qT[d, 128n+p] = q16[p, n, d] for chunks n in window h."""
                qtps = ps_qt.tile([P, 512], F16, tag="qt", name="qtps")
                for i in range(4):
                    n = 4 * h + i
                    nc.tensor.transpose(
                        qtps[:, i * P:(i + 1) * P], q16[:, n, :], ident
                    )
                return qtps

            def proj_win(qT, pp, h):
                """[Q;K].T window h: fp16 matmul -> fp32 PSUM."""
                cols = slice(h * 512, (h + 1) * 512)
                nc.tensor.matmul(
                    pp[:, cols], w16, qT[:, cols], start=True, stop=True
                )

            def stacks_win(pp, hb, lb, qstack, khh, h):
                """fp16 hi/lo of Q, hi of K, stacked, for window h."""
                cols = slice(h * 512, (h + 1) * 512)
                nc.vector.tensor_copy(hb[:, cols], pp[:, cols])
                nc.vector.tensor_tensor(
                    lb[:, cols], pp[0:DK, cols], hb[0:DK, cols], AL.subtract
                )
                nc.vector.tensor_copy(qstack[0:DK, cols], hb[0:DK, cols])
                nc.vector.tensor_copy(qstack[DK:P, cols], lb[:, cols])
                nc.vector.tensor_copy(khh[0:DK, cols], hb[DK:P, cols])
                nc.vector.tensor_copy(khh[DK:P, cols], hb[DK:P, cols])

            def prep_batch(q16):
                """transpose + proj + stacks, window-pipelined."""
                pp = ps_sc.tile([P, S], F32, tag="sc", name="pp")
                hb = projsb.tile([P, S], F16, tag="hb")    # [Qh; Kh]
                lb = projsb.tile([DK, S], F16, tag="lb")   # Ql
                qstack = projsb.tile([P, S], F16, tag="qstack")  # [Qh; Ql]
                khh = projsb.tile([P, S], F16, tag="khh")        # [Kh; Kh]
                qT = qtp.tile([P, S], F16, tag="qT")
                for h in range(2):
                    qtps = transpose_win(q16, h)
                    cols = slice(h * 512, (h + 1) * 512)
                    nc.vector.tensor_copy(qT[:, cols], qtps)
                    proj_win(qT, pp, h)
                    stacks_win(pp, hb, lb, qstack, khh, h)
                return qstack, khh

            def score_chunk(t16, qstack, khh, c):
                """One 128-row score chunk: 2 matmuls + diag accum + tanh."""
                sc = ps_sc.tile([P, S], F32, tag="sc", name="sc")
                hd = c // 4  # window containing this chunk's diag block
                for h in range(2):
                    cols = slice(h * 512, (h + 1) * 512)
                    nc.tensor.matmul(
                        sc[:, cols], qstack[:, c * P:(c + 1) * P],
                        khh[:, cols], start=True, stop=(h != hd),
                    )
                    if h == hd:
                        nc.tensor.matmul(
                            sc[:, c * P:(c + 1) * P], ident, negd,
                            start=False, stop=True, skip_group_check=True,
                        )
                nc.scalar.activation(
                    out=t16[:, c], in_=sc,
                    func=mybir.ActivationFunctionType.Tanh,
                )

            def exp_batch(t16, o32, rs):
                nc.scalar.activation(
                    out=o32, in_=t16,
                    func=mybir.ActivationFunctionType.Exp,
                    scale=TANH_CLIP,
                    accum_out=rs,
                )

            def zrecip(rs):
                zall = small.tile([P, 1], F32, tag="zall")
                nc.gpsimd.partition_all_reduce(
                    zall, rs, channels=P, reduce_op=bass_isa.ReduceOp.add
                )
                rz = small.tile([P, 1], F32, tag="rz")
                nc.vector.reciprocal(rz, zall)
                return rz

            def norm_store(b, o32, rz, ngrp):
                """Normalize + store in ngrp chunks so the store streams."""
                ov = out_d[b].rearrange("(n p) s -> p n s", p=P)
                w = NQ // ngrp
                for g in range(ngrp):
                    sl = slice(w * g, w * (g + 1))
                    nc.vector.tensor_scalar_mul(o32[:, sl], o32[:, sl], rz)
                    nc.sync.dma_start(ov[:, sl], o32[:, sl])

            # ---- software-pipelined batch loop ------------------------
            q16 = load(0)
            ops = prep_batch(q16)
            pending = None  # (b, o32, rz) awaiting normalize+store

            for b in range(BPC):
                t16 = tbuf.tile([P, NQ, S], F16, tag="t16")
                o32 = obuf.tile([P, NQ, S], F32, tag="o32")
                rs = small.tile([P, 1], F32, tag="rs")

                if b + 1 < BPC:
                    nq16 = load(b + 1)

                for c in range(NQ):
                    score_chunk(t16, *ops, c)

                if b + 1 < BPC:
                    ops = prep_batch(nq16)

                if pending is not None:
                    norm_store(*pending, ngrp=4)
                    pending = None

                exp_batch(t16, o32, rs)
                rz = zrecip(rs)
                pending = (b, o32, rz)

            # fine-grained tail: the last batch's store is the critical path
            norm_store(*pending, ngrp=8)

    nc.compile()
    return nc


_CACHED_NC = None


def kernel(**inputs: np.ndarray) -> np.ndarray:
    global _CACHED_NC
    query = np.ascontiguousarray(np.asarray(inputs["query"], dtype=np.float32))
    wq = np.ascontiguousarray(np.asarray(inputs["W_query"], dtype=np.float32))
    wk = np.ascontiguousarray(np.asarray(inputs["W_key"], dtype=np.float32))
    assert query.shape == (B, S, D), query.shape

    if _CACHED_NC is None:
        _CACHED_NC = build_bass()
    nc = _CACHED_NC

    in_maps = [
        {
            "query": query[c * BPC:(c + 1) * BPC],
            "W_query": wq,
            "W_key": wk,
        }
        for c in range(N_CORES)
    ]
    res = run_bass_kernel_spmd(nc, in_maps, core_ids=list(range(N_CORES)))
    out = np.concatenate(
        [r["out"].reshape(BPC, S * S) for r in res.results], axis=0
    )
    return out
